# revision 25
# baseline (speedup 1.0000x reference)
"""ALCOVE RBF similarity kernel for Trainium2 (8 NeuronCores, data-parallel).

Computes out[b, e] = exp(-C * sum_d attn[d] * |exemplars[e, d] - inputs[b, d]|)
for inputs (2048, 128), exemplars (2048, 128), attn (128,) -> out (2048, 2048).

Strategy (per core, batch shard of 256 rows):
  - Layout: d (=128) on SBUF partitions, ne (=2048) on the free axis.
  - Pre-scale exemplars/inputs by attn on device (attn_d * |e-x| == |attn_d*e - attn_d*x|
    since attn >= 0), exemplars cast to bf16.
  - VectorE: per batch row b, one fused tensor_scalar op computes
    |e'^T - x'_b| as abs_max(subtract(e', x'_b), 0) at 4x bf16 rate.
  - TensorE: reduce over d (partitions) via matmul with a ones-column stationary.
    4-way column tiling (tile_position=(0, 32j)) runs 4 batch rows concurrently;
    a sliding one-hot window in the stationary places row b at PSUM partition
    b % 128, accumulating over 32 quads into a (128, 2048) f32 PSUM tile.
  - ScalarE: exp(-C * dist) fused via activation scale, PSUM -> SBUF.
  - DMA the (128, 2048) f32 result rows back to DRAM.
"""

import numpy as np

import concourse.bacc as bacc
import concourse.tile as tile
from concourse import mybir
from concourse.bass_utils import run_bass_kernel_spmd


def _register_absdiff_op():
    """Register a custom DVE op TS_ABS_DIFF_ANT: out = |in0 - s0| in ONE
    VectorE instruction with full 1x/2x/4x perf-mode programs.

    The stock TENSOR_SCALAR uop programs (table slots 16..19 on gen3) are
    op-agnostic: their datapath stages use INSTRUCTION_OP_0/1 (wire 32/33),
    taking the ALU op from the instruction bytes. The cayman ISA has a float
    ABSOLUTE_DIFF ALU op (wire 23), but bass's AluOpType cannot express it.
    So we clone the stock programs with the ALU op hardwired to
    ABSOLUTE_DIFF (op1 stage -> BYPASS) and register them as a custom DVE op
    with perf_max=3, inheriting the stock modes' packing/port wiring.
    """
    import concourse.dve_ops as dve_ops
    from concourse.dve_spec import Spec, Src0, C0, Bin, AluOp as DAluOp
    from concourse.dve_uop import DveOpSpec
    from concourse.dve_tables import find_stock_dve_bin_dir, load_table_set

    NAME = "TS_ABS_DIFF_ANT"
    for op in dve_ops.OPS:
        if op.name == NAME:
            return op

    ts = load_table_set(find_stock_dve_bin_dir("gen3"), "default", "v3")

    class RawUop:
        next_uop = (0, 0, 0)

        def __init__(self, slot):
            self.cf = dict(ts.control_fast[slot])
            self.cs = dict(ts.control_slow[slot])
            self.dp = [dict(b) for b in ts.datapath[slot]]
            for b in self.dp:
                if b.get("alu_op") == 32:      # INSTRUCTION_OP_0 -> ABSOLUTE_DIFF
                    b["alu_op"] = 23
                elif b.get("alu_op") == 33:    # INSTRUCTION_OP_1 -> BYPASS
                    b["alu_op"] = 0

        def to_entries(self, ver):
            return (self.cf, self.cs, self.dp)

        def validate(self, ver):
            pass

    spec = Spec(
        body=Bin(DAluOp.ABSOLUTE_DIFF, Src0, C0),
        reference=lambda in0, in1, s0, s1, imm2: np.abs(in0 - s0),
    )

    row = dve_ops._CUSTOM_DVE_ROW_BASE + len(dve_ops.OPS)
    assert row < 0x20
    compiled = DveOpSpec(
        name=NAME,
        opcode=row,
        uops=[RawUop(16)],
        uops_2x=[RawUop(17)],
        uops_2x_2p=[RawUop(18)],
        uops_4x=[RawUop(19)],
        perf_max=3,
        rd1_en=False,
    )

    class AbsDiffOp:
        pass

    op = AbsDiffOp()
    op.name = NAME
    op.spec = spec
    op.subdim = False
    op.compile = lambda ver, _c=compiled: _c
    dve_ops.OPS.append(op)
    dve_ops._SUB_OPCODE_FOR_NAME[NAME] = row
    dve_ops.CUSTOM_DVE_SPECS[NAME] = spec
    return op


ABS_DIFF_OP = _register_absdiff_op()

N_CORES = 8
BATCH = 2048
NE = 2048
ND = 128
BL = BATCH // N_CORES  # 256 batch rows per core
C = 6.5

F32 = mybir.dt.float32
BF16 = mybir.dt.bfloat16
AF = mybir.ActivationFunctionType
OP = mybir.AluOpType

N_GROUPS = BL // 128          # 2 groups of 128 batch rows
N_QUADS = 32                  # quads per group (4 rows each)
N_CHUNKS = NE // 512          # 4 matmul chunks of 512 cols

# Fraction of batch rows whose abs-diff tile is produced on ScalarE
# (activation Abs with per-partition bias, measured ~1894ns) vs VectorE
# (custom fused |in0 - s0| at 4x bf16, measured ~663ns). Balanced so both
# engines finish together (ScalarE also runs the exp epilogues).
ACT_FRACTION = 663.0 / (663.0 + 1894.0)


def build_nc():
    nc = bacc.Bacc(
        "TRN2", target_bir_lowering=False, debug=False, num_devices=N_CORES
    )

    xT = nc.dram_tensor("xT", [ND, BL], F32, kind="ExternalInput")
    eT = nc.dram_tensor("eT", [ND, NE], F32, kind="ExternalInput")
    attn = nc.dram_tensor("attn", [ND, 1], F32, kind="ExternalInput")
    out = nc.dram_tensor("out", [BL, NE], F32, kind="ExternalOutput")

    with tile.TileContext(nc) as tc:
        with (
            tc.tile_pool(name="const", bufs=1) as cpool,
            tc.tile_pool(name="adp", bufs=10) as adpool,
            tc.tile_pool(name="psum", bufs=2, space="PSUM") as ppool,
            tc.tile_pool(name="outp", bufs=2) as opool,
        ):
            eT_sb = cpool.tile([ND, NE], F32)
            e_bf = cpool.tile([ND, NE], BF16)
            xT_sb = cpool.tile([ND, BL], F32)
            xs = cpool.tile([ND, BL], F32)      # attn-scaled inputs (f32)
            nxs = cpool.tile([ND, BL], F32)     # -attn-scaled inputs (bias form)
            at_sb = cpool.tile([ND, 1], F32)
            W = cpool.tile([ND, 63], BF16)      # sliding one-hot ones column
            Z = cpool.tile([ND, 128], BF16)     # all-zero stationary (PSUM clear)

            for c in range(4):
                nc.sync.dma_start(
                    out=eT_sb[:, 512 * c:512 * (c + 1)],
                    in_=eT[:, 512 * c:512 * (c + 1)],
                )
            nc.sync.dma_start(out=xT_sb[:, :], in_=xT[:, :])
            nc.sync.dma_start(out=at_sb[:, :], in_=attn[:, :])

            nc.vector.memset(W[:, :], 0.0)
            nc.vector.memset(W[:, 31:32], 1.0)
            nc.vector.memset(Z[:, :], 0.0)

            # Prime the ScalarE activation tables (Exp set, which also holds
            # Abs) while the input DMAs are still in flight, so the ~2.7us
            # table load is off the critical path.
            prime = cpool.tile([ND, 1], F32)
            nc.vector.memset(prime[:, :], 0.0)
            nc.scalar.activation(
                out=prime[:, :], in_=prime[:, :], func=AF.Exp,
                bias=0.0, scale=1.0,
            )
            nc.scalar.activation(
                out=prime[:, :], in_=prime[:, :], func=AF.Abs,
                bias=0.0, scale=1.0,
            )

            # e' = attn * e (bf16), x' = attn * x (f32), -x' for ScalarE bias
            for c in range(4):
                nc.vector.tensor_scalar(
                    out=e_bf[:, 512 * c:512 * (c + 1)],
                    in0=eT_sb[:, 512 * c:512 * (c + 1)],
                    scalar1=at_sb[:, 0:1],
                    scalar2=None, op0=OP.mult,
                )
            nc.vector.tensor_scalar(
                out=xs[:, :], in0=xT_sb[:, :], scalar1=at_sb[:, 0:1],
                scalar2=None, op0=OP.mult,
            )
            nc.vector.tensor_scalar(
                out=nxs[:, :], in0=xs[:, :], scalar1=-1.0,
                scalar2=None, op0=OP.mult,
            )

            for g in range(N_GROUPS):
                psum_t = ppool.tile([128, NE], F32)
                # Clear each PSUM bank: full-partition zero matmul opens the
                # accumulation group (start=True zeroes has_written + data).
                for c in range(N_CHUNKS):
                    nc.tensor.matmul(
                        out=psum_t[:, 512 * c:512 * (c + 1)],
                        lhsT=Z[:, :],
                        rhs=e_bf[:, 0:512],
                        start=True,
                        stop=False,
                        tile_position=(0, 0),
                    )
                for q in range(N_QUADS):
                    ad_tiles = []
                    for j in range(4):
                        b = 128 * g + 32 * j + q
                        ad = adpool.tile([ND, NE], BF16, tag="ad")
                        # Skew ScalarE's share away from the first quads (so
                        # TensorE can start without waiting on ScalarE's first
                        # op) and the last quads (so the exp epilogue isn't
                        # gated by a late ScalarE abs-diff).
                        k = 128 * g + 4 * q + j
                        af = ACT_FRACTION * 128.0 / 112.0
                        act_cnt = (
                            int((k + 1) * af) - int(k * af)
                            if 8 <= k % 128 < 120 else 0
                        )
                        if act_cnt:
                            # ScalarE: |e' - x'_b| = Abs(e' + (-x'_b))
                            nc.scalar.activation(
                                out=ad[:, :], in_=e_bf[:, :], func=AF.Abs,
                                bias=nxs[:, b:b + 1], scale=1.0,
                            )
                        else:
                            # VectorE: custom fused |e' - x'_b| (ABSOLUTE_DIFF
                            # ALU op) with stock tensor_scalar perf modes.
                            r = nc.vector._custom_dve(
                                ABS_DIFF_OP, out=ad[:, :], in0=e_bf[:, :],
                                s0=xs[:, b:b + 1],
                            )
                            r.ins.perf_max = 3
                        ad_tiles.append(ad)
                    for j in range(4):
                        lhsT = W[:, 31 - q:63 - q]
                        for c in range(N_CHUNKS):
                            nc.tensor.matmul(
                                out=psum_t[32 * j:32 * j + 32, 512 * c:512 * (c + 1)],
                                lhsT=lhsT,
                                rhs=ad_tiles[j][:, 512 * c:512 * (c + 1)],
                                start=False,
                                stop=False,
                                tile_position=(0, 32 * j),
                            )
                # Close each bank's accumulation group with a full-partition
                # zero matmul (adds 0; start=False keeps data, stop=True
                # closes the group over all 128 partitions).
                for c in range(N_CHUNKS):
                    nc.tensor.matmul(
                        out=psum_t[:, 512 * c:512 * (c + 1)],
                        lhsT=Z[:, :],
                        rhs=e_bf[:, 0:512],
                        start=False,
                        stop=True,
                        tile_position=(0, 0),
                    )

                out_t = opool.tile([128, NE], F32, tag="ot")
                for c in range(N_CHUNKS):
                    nc.scalar.activation(
                        out=out_t[:, 512 * c:512 * (c + 1)],
                        in_=psum_t[:, 512 * c:512 * (c + 1)],
                        func=AF.Exp, bias=0.0, scale=-C,
                    )
                    nc.sync.dma_start(
                        out=out[128 * g:128 * (g + 1), 512 * c:512 * (c + 1)],
                        in_=out_t[:, 512 * c:512 * (c + 1)],
                    )

    nc.compile()
    return nc


_NC_CACHE = None


def _get_nc():
    global _NC_CACHE
    if _NC_CACHE is None:
        _NC_CACHE = build_nc()
    return _NC_CACHE


def make_in_maps(inputs, exemplars, attn):
    eT = np.ascontiguousarray(exemplars.astype(np.float32).T)          # (128, 2048)
    at = np.ascontiguousarray(attn.astype(np.float32).reshape(ND, 1))  # (128, 1)
    in_maps = []
    for i in range(N_CORES):
        xT = np.ascontiguousarray(
            inputs[i * BL:(i + 1) * BL].astype(np.float32).T
        )  # (128, 256)
        in_maps.append({"xT": xT, "eT": eT, "attn": at})
    return in_maps


def run(inputs, exemplars, attn, trace=False, **kwargs):
    nc = _get_nc()
    in_maps = make_in_maps(np.asarray(inputs), np.asarray(exemplars), np.asarray(attn))
    res = run_bass_kernel_spmd(
        nc, in_maps, core_ids=list(range(N_CORES)), trace=trace, **kwargs
    )
    out = np.concatenate([res.results[i]["out"] for i in range(N_CORES)], axis=0)
    return out, res


def kernel(inputs, exemplars, attn):
    out, _ = run(inputs, exemplars, attn, trace=False)
    return out


# revision 42
# speedup vs baseline: 1.0529x; 1.0529x over previous
"""ALCOVE RBF similarity kernel for Trainium2 (8 NeuronCores, data-parallel).

Computes out[b, e] = exp(-C * sum_d attn[d] * |exemplars[e, d] - inputs[b, d]|)
for inputs (2048, 128), exemplars (2048, 128), attn (128,) -> out (2048, 2048).

Strategy (per core, batch shard of 256 rows):
  - Layout: d (=128) on SBUF partitions, ne (=2048) on the free axis.
  - Pre-scale exemplars/inputs by attn on device (attn_d * |e-x| == |attn_d*e - attn_d*x|
    since attn >= 0), exemplars cast to bf16.
  - VectorE: per batch row b, one fused tensor_scalar op computes
    |e'^T - x'_b| as abs_max(subtract(e', x'_b), 0) at 4x bf16 rate.
  - TensorE: reduce over d (partitions) via matmul with a ones-column stationary.
    4-way column tiling (tile_position=(0, 32j)) runs 4 batch rows concurrently;
    a sliding one-hot window in the stationary places row b at PSUM partition
    b % 128, accumulating over 32 quads into a (128, 2048) f32 PSUM tile.
  - ScalarE: exp(-C * dist) fused via activation scale, PSUM -> SBUF.
  - DMA the (128, 2048) f32 result rows back to DRAM.
"""

import numpy as np

import concourse.bacc as bacc
import concourse.tile as tile
from concourse import mybir
from concourse.bass_utils import run_bass_kernel_spmd


def _register_absdiff_op():
    """Register a custom DVE op TS_ABS_DIFF_ANT: out = |in0 - s0| in ONE
    VectorE instruction with full 1x/2x/4x perf-mode programs.

    The stock TENSOR_SCALAR uop programs (table slots 16..19 on gen3) are
    op-agnostic: their datapath stages use INSTRUCTION_OP_0/1 (wire 32/33),
    taking the ALU op from the instruction bytes. The cayman ISA has a float
    ABSOLUTE_DIFF ALU op (wire 23), but bass's AluOpType cannot express it.
    So we clone the stock programs with the ALU op hardwired to
    ABSOLUTE_DIFF (op1 stage -> BYPASS) and register them as a custom DVE op
    with perf_max=3, inheriting the stock modes' packing/port wiring.
    """
    import concourse.dve_ops as dve_ops
    from concourse.dve_spec import Spec, Src0, C0, Bin, AluOp as DAluOp
    from concourse.dve_uop import DveOpSpec
    from concourse.dve_tables import find_stock_dve_bin_dir, load_table_set

    NAME = "TS_ABS_DIFF_ANT"
    for op in dve_ops.OPS:
        if op.name == NAME:
            return op

    ts = load_table_set(find_stock_dve_bin_dir("gen3"), "default", "v3")

    class RawUop:
        next_uop = (0, 0, 0)

        def __init__(self, slot):
            self.cf = dict(ts.control_fast[slot])
            self.cs = dict(ts.control_slow[slot])
            self.dp = [dict(b) for b in ts.datapath[slot]]
            for b in self.dp:
                if b.get("alu_op") == 32:      # INSTRUCTION_OP_0 -> ABSOLUTE_DIFF
                    b["alu_op"] = 23
                elif b.get("alu_op") == 33:    # INSTRUCTION_OP_1 -> BYPASS
                    b["alu_op"] = 0

        def to_entries(self, ver):
            return (self.cf, self.cs, self.dp)

        def validate(self, ver):
            pass

    spec = Spec(
        body=Bin(DAluOp.ABSOLUTE_DIFF, Src0, C0),
        reference=lambda in0, in1, s0, s1, imm2: np.abs(in0 - s0),
    )

    row = dve_ops._CUSTOM_DVE_ROW_BASE + len(dve_ops.OPS)
    assert row < 0x20
    compiled = DveOpSpec(
        name=NAME,
        opcode=row,
        uops=[RawUop(16)],
        uops_2x=[RawUop(17)],
        uops_2x_2p=[RawUop(18)],
        uops_4x=[RawUop(19)],
        perf_max=3,
        rd1_en=False,
    )

    class AbsDiffOp:
        pass

    op = AbsDiffOp()
    op.name = NAME
    op.spec = spec
    op.subdim = False
    op.compile = lambda ver, _c=compiled: _c
    dve_ops.OPS.append(op)
    dve_ops._SUB_OPCODE_FOR_NAME[NAME] = row
    dve_ops.CUSTOM_DVE_SPECS[NAME] = spec
    return op


ABS_DIFF_OP = _register_absdiff_op()

N_CORES = 8
BATCH = 2048
NE = 2048
ND = 128
BL = BATCH // N_CORES  # 256 batch rows per core
C = 6.5

F32 = mybir.dt.float32
BF16 = mybir.dt.bfloat16
AF = mybir.ActivationFunctionType
OP = mybir.AluOpType

N_GROUPS = BL // 128          # 2 groups of 128 batch rows
N_QUADS = 32                  # quads per group (4 rows each)
N_CHUNKS = NE // 512          # 4 matmul chunks of 512 cols

# Fraction of batch rows whose abs-diff tile is produced on ScalarE
# (activation Abs with per-partition bias, measured ~1894ns) vs VectorE
# (custom fused |in0 - s0| at 4x bf16, measured ~663ns). Balanced so both
# engines finish together (ScalarE also runs the exp epilogues).
ACT_FRACTION = 0.275  # 66 of 240 eligible rows on ScalarE (measured balance)


def build_nc():
    nc = bacc.Bacc(
        "TRN2", target_bir_lowering=False, debug=False, num_devices=N_CORES
    )

    xT = nc.dram_tensor("xT", [ND, BL], F32, kind="ExternalInput")
    eT = nc.dram_tensor("eT", [ND, NE], F32, kind="ExternalInput")
    attn = nc.dram_tensor("attn", [ND, 1], F32, kind="ExternalInput")
    out = nc.dram_tensor("out", [BL, NE], F32, kind="ExternalOutput")

    with tile.TileContext(nc) as tc:
        with (
            tc.tile_pool(name="const", bufs=1) as cpool,
            tc.tile_pool(name="adp", bufs=12) as adpool,
            tc.tile_pool(name="psum", bufs=2, space="PSUM") as ppool,
            tc.tile_pool(name="outp", bufs=2) as opool,
        ):
            eT_sb = cpool.tile([ND, NE], F32)
            e_bf = cpool.tile([ND, NE], BF16)
            xT_sb = cpool.tile([ND, BL], F32)
            xs = cpool.tile([ND, BL], F32)      # attn-scaled inputs (f32)
            nxs = cpool.tile([ND, BL], F32)     # -attn-scaled inputs (bias form)
            at_sb = cpool.tile([ND, 1], F32)
            W = cpool.tile([ND, 63], BF16)      # sliding one-hot ones column
            Z = cpool.tile([ND, 128], BF16)     # all-zero stationary (PSUM clear)

            # Spread the 1MB exemplar load over DMA queues AND over three
            # issuing engines: descriptor generation costs ~0.7us per
            # dma_start on the issuing sequencer, so a single engine would
            # serialize ~11us of issue latency.
            issuers = [nc.sync, nc.gpsimd]
            nc.gpsimd.dma_start(out=at_sb[:, :], in_=attn[:, :])
            nc.sync.dma_start(out=xT_sb[:, :], in_=xT[:, :])
            for c in range(12):
                issuers[c % 2].dma_start(
                    out=eT_sb[:, 170 * c:170 * (c + 1) if c < 11 else NE],
                    in_=eT[:, 170 * c:170 * (c + 1) if c < 11 else NE],
                )

            nc.vector.memset(W[:, :], 0.0)
            nc.vector.memset(W[:, 31:32], 1.0)
            nc.vector.memset(Z[:, :], 0.0)

            # Prime the ScalarE activation tables (Exp set, which also holds
            # Abs) while the input DMAs are still in flight, so the ~2.7us
            # table load is off the critical path.
            prime = cpool.tile([ND, 1], F32)
            nc.vector.memset(prime[:, :], 0.0)
            nc.scalar.activation(
                out=prime[:, :], in_=prime[:, :], func=AF.Exp,
                bias=0.0, scale=1.0,
            )
            nc.scalar.activation(
                out=prime[:, :], in_=prime[:, :], func=AF.Abs,
                bias=0.0, scale=1.0,
            )

            # Prime the custom-DVE uop table the same way: a dummy fused
            # abs-diff op issued before the inputs arrive pulls the per-NEFF
            # DVE table load off the critical path.
            vprime = cpool.tile([ND, 2], BF16)
            vprime_s = cpool.tile([ND, 1], F32)
            nc.vector.memset(vprime[:, :], 0.0)
            nc.vector.memset(vprime_s[:, :], 0.0)
            r = nc.vector._custom_dve(
                ABS_DIFF_OP, out=vprime[:, :], in0=vprime[:, :],
                s0=vprime_s[:, 0:1],
            )
            r.ins.perf_max = 3

            # x' = attn * x (f32) and -x' first: they only need the tiny xT
            # DMA, so ScalarE's first abs-diff isn't gated on the big load.
            nc.vector.tensor_scalar(
                out=xs[:, :], in0=xT_sb[:, :], scalar1=at_sb[:, 0:1],
                scalar2=None, op0=OP.mult,
            )
            nc.vector.tensor_scalar(
                out=nxs[:, :], in0=xs[:, :], scalar1=-1.0,
                scalar2=None, op0=OP.mult,
            )
            # e' = attn * e (bf16)
            for c in range(4):
                nc.vector.tensor_scalar(
                    out=e_bf[:, 512 * c:512 * (c + 1)],
                    in0=eT_sb[:, 512 * c:512 * (c + 1)],
                    scalar1=at_sb[:, 0:1],
                    scalar2=None, op0=OP.mult,
                )

            for g in range(N_GROUPS):
                psum_t = ppool.tile([128, NE], F32)
                # Clear each PSUM bank: full-partition zero matmul opens the
                # accumulation group (start=True zeroes has_written + data).
                for c in range(N_CHUNKS):
                    nc.tensor.matmul(
                        out=psum_t[:, 512 * c:512 * (c + 1)],
                        lhsT=Z[:, :],
                        rhs=e_bf[:, 0:512],
                        start=True,
                        stop=False,
                        tile_position=(0, 0),
                    )
                for q in range(N_QUADS):
                    ad_tiles = []
                    for j in range(4):
                        b = 128 * g + 32 * j + q
                        ad = adpool.tile([ND, NE], BF16, tag="ad")
                        # Give the last quads of each group to VectorE so the
                        # exp epilogue isn't gated by a late ScalarE abs-diff.
                        k = 128 * g + 4 * q + j
                        act_cnt = (
                            int((k + 1) * ACT_FRACTION) - int(k * ACT_FRACTION)
                            if k % 128 < 120 else 0
                        )
                        if act_cnt:
                            # ScalarE: |e' - x'_b| = Abs(e' + (-x'_b))
                            nc.scalar.activation(
                                out=ad[:, :], in_=e_bf[:, :], func=AF.Abs,
                                bias=nxs[:, b:b + 1], scale=1.0,
                            )
                        else:
                            # VectorE: custom fused |e' - x'_b| (ABSOLUTE_DIFF
                            # ALU op) with stock tensor_scalar perf modes.
                            r = nc.vector._custom_dve(
                                ABS_DIFF_OP, out=ad[:, :], in0=e_bf[:, :],
                                s0=xs[:, b:b + 1],
                            )
                            r.ins.perf_max = 3
                        ad_tiles.append(ad)
                    for j in range(4):
                        lhsT = W[:, 31 - q:63 - q]
                        for c in range(N_CHUNKS):
                            nc.tensor.matmul(
                                out=psum_t[32 * j:32 * j + 32, 512 * c:512 * (c + 1)],
                                lhsT=lhsT,
                                rhs=ad_tiles[j][:, 512 * c:512 * (c + 1)],
                                start=False,
                                stop=False,
                                tile_position=(0, 32 * j),
                            )
                # Close each bank's accumulation group with a full-partition
                # zero matmul (adds 0; start=False keeps data, stop=True
                # closes the group over all 128 partitions). N=1: the stop
                # applies to the whole bank region.
                for c in range(N_CHUNKS):
                    nc.tensor.matmul(
                        out=psum_t[:, 512 * c:512 * c + 1],
                        lhsT=Z[:, :],
                        rhs=e_bf[:, 0:1],
                        start=False,
                        stop=True,
                        tile_position=(0, 0),
                    )

                out_t = opool.tile([128, NE], F32, tag="ot")
                for c in range(N_CHUNKS):
                    nc.scalar.activation(
                        out=out_t[:, 512 * c:512 * (c + 1)],
                        in_=psum_t[:, 512 * c:512 * (c + 1)],
                        func=AF.Exp, bias=0.0, scale=-C,
                    )
                    # Alternate the issuing engine: descriptor generation is
                    # ~0.7us on the issuing sequencer.
                    (nc.sync if c % 2 == 0 else nc.gpsimd).dma_start(
                        out=out[128 * g:128 * (g + 1), 512 * c:512 * (c + 1)],
                        in_=out_t[:, 512 * c:512 * (c + 1)],
                    )

    nc.compile()
    return nc


_NC_CACHE = None


def _get_nc():
    global _NC_CACHE
    if _NC_CACHE is None:
        _NC_CACHE = build_nc()
    return _NC_CACHE


def make_in_maps(inputs, exemplars, attn):
    eT = np.ascontiguousarray(exemplars.astype(np.float32).T)          # (128, 2048)
    at = np.ascontiguousarray(attn.astype(np.float32).reshape(ND, 1))  # (128, 1)
    in_maps = []
    for i in range(N_CORES):
        xT = np.ascontiguousarray(
            inputs[i * BL:(i + 1) * BL].astype(np.float32).T
        )  # (128, 256)
        in_maps.append({"xT": xT, "eT": eT, "attn": at})
    return in_maps


def run(inputs, exemplars, attn, trace=False, **kwargs):
    nc = _get_nc()
    in_maps = make_in_maps(np.asarray(inputs), np.asarray(exemplars), np.asarray(attn))
    res = run_bass_kernel_spmd(
        nc, in_maps, core_ids=list(range(N_CORES)), trace=trace, **kwargs
    )
    out = np.concatenate([res.results[i]["out"] for i in range(N_CORES)], axis=0)
    return out, res


def kernel(inputs, exemplars, attn):
    out, _ = run(inputs, exemplars, attn, trace=False)
    return out


# revision 43
# speedup vs baseline: 1.0600x; 1.0067x over previous
"""ALCOVE RBF similarity kernel for Trainium2 (8 NeuronCores, data-parallel).

Computes out[b, e] = exp(-C * sum_d attn[d] * |exemplars[e, d] - inputs[b, d]|)
for inputs (2048, 128), exemplars (2048, 128), attn (128,) -> out (2048, 2048).

Strategy (per core, batch shard of 256 rows):
  - Layout: d (=128) on SBUF partitions, ne (=2048) on the free axis.
  - Pre-scale exemplars/inputs by attn on device (attn_d * |e-x| == |attn_d*e - attn_d*x|
    since attn >= 0), exemplars cast to bf16.
  - Abs-diff tiles |e'^T - x'_b| (128 x 2048 bf16, one per batch row) are
    produced by BOTH elementwise engines, balanced so they finish together:
      * VectorE (~74%): a custom DVE op (registered below) that runs the
        cayman float ABSOLUTE_DIFF ALU op with the stock tensor_scalar
        perf-mode programs -> 4x bf16 rate, one op per row (~663ns).
      * ScalarE (~26%): activation(Abs, bias=-x'_b per partition) (~1894ns).
  - TensorE: reduce over d (partitions) via matmul with a ones-column
    stationary. 4-way column tiling (tile_position=(0, 32j)) runs 4 batch
    rows concurrently; a sliding one-hot window in the stationary places row
    b at PSUM partition b % 128, accumulating over 32 quads into a
    (128, 2048) f32 PSUM tile (opened/closed by zero-matmuls carrying
    start/stop).
  - ScalarE: exp(-C * dist) fused via activation scale, PSUM -> SBUF, chunked
    so output DMA overlaps.
  - DMA issue is spread across the idle sync/gpsimd sequencers (descriptor
    generation costs ~0.7us per dma_start on the issuing engine).
  - Whole-chip: 8-way batch-parallel, no collectives; host transposes inputs
    and concatenates the 8 output shards.

Measured on TRN2: ~155us NEFF exec (fast clock state; the chip sometimes
sits in a ~1.2x slower power state), rel err ~9e-4 vs the f32 reference.
VectorE/ScalarE are both >95% busy in steady state; TensorE ~27%.
"""

import numpy as np

import concourse.bacc as bacc
import concourse.tile as tile
from concourse import mybir
from concourse.bass_utils import run_bass_kernel_spmd


def _register_absdiff_op():
    """Register a custom DVE op TS_ABS_DIFF_ANT: out = |in0 - s0| in ONE
    VectorE instruction with full 1x/2x/4x perf-mode programs.

    The stock TENSOR_SCALAR uop programs (table slots 16..19 on gen3) are
    op-agnostic: their datapath stages use INSTRUCTION_OP_0/1 (wire 32/33),
    taking the ALU op from the instruction bytes. The cayman ISA has a float
    ABSOLUTE_DIFF ALU op (wire 23), but bass's AluOpType cannot express it.
    So we clone the stock programs with the ALU op hardwired to
    ABSOLUTE_DIFF (op1 stage -> BYPASS) and register them as a custom DVE op
    with perf_max=3, inheriting the stock modes' packing/port wiring.
    """
    import concourse.dve_ops as dve_ops
    from concourse.dve_spec import Spec, Src0, C0, Bin, AluOp as DAluOp
    from concourse.dve_uop import DveOpSpec
    from concourse.dve_tables import find_stock_dve_bin_dir, load_table_set

    NAME = "TS_ABS_DIFF_ANT"
    for op in dve_ops.OPS:
        if op.name == NAME:
            return op

    ts = load_table_set(find_stock_dve_bin_dir("gen3"), "default", "v3")

    class RawUop:
        next_uop = (0, 0, 0)

        def __init__(self, slot):
            self.cf = dict(ts.control_fast[slot])
            self.cs = dict(ts.control_slow[slot])
            self.dp = [dict(b) for b in ts.datapath[slot]]
            for b in self.dp:
                if b.get("alu_op") == 32:      # INSTRUCTION_OP_0 -> ABSOLUTE_DIFF
                    b["alu_op"] = 23
                elif b.get("alu_op") == 33:    # INSTRUCTION_OP_1 -> BYPASS
                    b["alu_op"] = 0

        def to_entries(self, ver):
            return (self.cf, self.cs, self.dp)

        def validate(self, ver):
            pass

    spec = Spec(
        body=Bin(DAluOp.ABSOLUTE_DIFF, Src0, C0),
        reference=lambda in0, in1, s0, s1, imm2: np.abs(in0 - s0),
    )

    row = dve_ops._CUSTOM_DVE_ROW_BASE + len(dve_ops.OPS)
    assert row < 0x20
    compiled = DveOpSpec(
        name=NAME,
        opcode=row,
        uops=[RawUop(16)],
        uops_2x=[RawUop(17)],
        uops_2x_2p=[RawUop(18)],
        uops_4x=[RawUop(19)],
        perf_max=3,
        rd1_en=False,
    )

    class AbsDiffOp:
        pass

    op = AbsDiffOp()
    op.name = NAME
    op.spec = spec
    op.subdim = False
    op.compile = lambda ver, _c=compiled: _c
    dve_ops.OPS.append(op)
    dve_ops._SUB_OPCODE_FOR_NAME[NAME] = row
    dve_ops.CUSTOM_DVE_SPECS[NAME] = spec
    return op


ABS_DIFF_OP = _register_absdiff_op()

N_CORES = 8
BATCH = 2048
NE = 2048
ND = 128
BL = BATCH // N_CORES  # 256 batch rows per core
C = 6.5

F32 = mybir.dt.float32
BF16 = mybir.dt.bfloat16
AF = mybir.ActivationFunctionType
OP = mybir.AluOpType

N_GROUPS = BL // 128          # 2 groups of 128 batch rows
N_QUADS = 32                  # quads per group (4 rows each)
N_CHUNKS = NE // 512          # 4 matmul chunks of 512 cols

# Fraction of batch rows whose abs-diff tile is produced on ScalarE
# (activation Abs with per-partition bias, measured ~1894ns) vs VectorE
# (custom fused |in0 - s0| at 4x bf16, measured ~663ns). Balanced so both
# engines finish together (ScalarE also runs the exp epilogues).
ACT_FRACTION = 0.275  # 66 of 240 eligible rows on ScalarE (measured balance)


def build_nc():
    nc = bacc.Bacc(
        "TRN2", target_bir_lowering=False, debug=False, num_devices=N_CORES
    )

    xT = nc.dram_tensor("xT", [ND, BL], F32, kind="ExternalInput")
    eT = nc.dram_tensor("eT", [ND, NE], F32, kind="ExternalInput")
    attn = nc.dram_tensor("attn", [ND, 1], F32, kind="ExternalInput")
    out = nc.dram_tensor("out", [BL, NE], F32, kind="ExternalOutput")

    with tile.TileContext(nc) as tc:
        with (
            tc.tile_pool(name="const", bufs=1) as cpool,
            tc.tile_pool(name="adp", bufs=12) as adpool,
            tc.tile_pool(name="psum", bufs=2, space="PSUM") as ppool,
            tc.tile_pool(name="outp", bufs=2) as opool,
        ):
            eT_sb = cpool.tile([ND, NE], F32)
            e_bf = cpool.tile([ND, NE], BF16)
            xT_sb = cpool.tile([ND, BL], F32)
            xs = cpool.tile([ND, BL], F32)      # attn-scaled inputs (f32)
            nxs = cpool.tile([ND, BL], F32)     # -attn-scaled inputs (bias form)
            at_sb = cpool.tile([ND, 1], F32)
            W = cpool.tile([ND, 63], BF16)      # sliding one-hot ones column
            Z = cpool.tile([ND, 128], BF16)     # all-zero stationary (PSUM clear)

            # Spread the 1MB exemplar load over DMA queues AND over three
            # issuing engines: descriptor generation costs ~0.7us per
            # dma_start on the issuing sequencer, so a single engine would
            # serialize ~11us of issue latency.
            issuers = [nc.sync, nc.gpsimd]
            nc.gpsimd.dma_start(out=at_sb[:, :], in_=attn[:, :])
            nc.sync.dma_start(out=xT_sb[:, :], in_=xT[:, :])
            for c in range(12):
                issuers[c % 2].dma_start(
                    out=eT_sb[:, 170 * c:170 * (c + 1) if c < 11 else NE],
                    in_=eT[:, 170 * c:170 * (c + 1) if c < 11 else NE],
                )

            nc.vector.memset(W[:, :], 0.0)
            nc.vector.memset(W[:, 31:32], 1.0)
            nc.vector.memset(Z[:, :], 0.0)

            # Prime the ScalarE activation tables (Exp set, which also holds
            # Abs) while the input DMAs are still in flight, so the ~2.7us
            # table load is off the critical path.
            prime = cpool.tile([ND, 1], F32)
            nc.vector.memset(prime[:, :], 0.0)
            nc.scalar.activation(
                out=prime[:, :], in_=prime[:, :], func=AF.Exp,
                bias=0.0, scale=1.0,
            )
            nc.scalar.activation(
                out=prime[:, :], in_=prime[:, :], func=AF.Abs,
                bias=0.0, scale=1.0,
            )

            # Prime the custom-DVE uop table the same way: a dummy fused
            # abs-diff op issued before the inputs arrive pulls the per-NEFF
            # DVE table load off the critical path.
            vprime = cpool.tile([ND, 2], BF16)
            vprime_s = cpool.tile([ND, 1], F32)
            nc.vector.memset(vprime[:, :], 0.0)
            nc.vector.memset(vprime_s[:, :], 0.0)
            r = nc.vector._custom_dve(
                ABS_DIFF_OP, out=vprime[:, :], in0=vprime[:, :],
                s0=vprime_s[:, 0:1],
            )
            r.ins.perf_max = 3

            # x' = attn * x (f32) and -x' first: they only need the tiny xT
            # DMA, so ScalarE's first abs-diff isn't gated on the big load.
            nc.vector.tensor_scalar(
                out=xs[:, :], in0=xT_sb[:, :], scalar1=at_sb[:, 0:1],
                scalar2=None, op0=OP.mult,
            )
            nc.vector.tensor_scalar(
                out=nxs[:, :], in0=xs[:, :], scalar1=-1.0,
                scalar2=None, op0=OP.mult,
            )
            # e' = attn * e (bf16)
            for c in range(4):
                nc.vector.tensor_scalar(
                    out=e_bf[:, 512 * c:512 * (c + 1)],
                    in0=eT_sb[:, 512 * c:512 * (c + 1)],
                    scalar1=at_sb[:, 0:1],
                    scalar2=None, op0=OP.mult,
                )

            for g in range(N_GROUPS):
                psum_t = ppool.tile([128, NE], F32)
                # Clear each PSUM bank: full-partition zero matmul opens the
                # accumulation group (start=True zeroes has_written + data).
                for c in range(N_CHUNKS):
                    nc.tensor.matmul(
                        out=psum_t[:, 512 * c:512 * (c + 1)],
                        lhsT=Z[:, :],
                        rhs=e_bf[:, 0:512],
                        start=True,
                        stop=False,
                        tile_position=(0, 0),
                    )
                for q in range(N_QUADS):
                    ad_tiles = []
                    for j in range(4):
                        b = 128 * g + 32 * j + q
                        ad = adpool.tile([ND, NE], BF16, tag="ad")
                        # Give the last quads of each group to VectorE so the
                        # exp epilogue isn't gated by a late ScalarE abs-diff.
                        k = 128 * g + 4 * q + j
                        act_cnt = (
                            int((k + 1) * ACT_FRACTION) - int(k * ACT_FRACTION)
                            if k % 128 < 120 else 0
                        )
                        if act_cnt:
                            # ScalarE: |e' - x'_b| = Abs(e' + (-x'_b))
                            nc.scalar.activation(
                                out=ad[:, :], in_=e_bf[:, :], func=AF.Abs,
                                bias=nxs[:, b:b + 1], scale=1.0,
                            )
                        else:
                            # VectorE: custom fused |e' - x'_b| (ABSOLUTE_DIFF
                            # ALU op) with stock tensor_scalar perf modes.
                            r = nc.vector._custom_dve(
                                ABS_DIFF_OP, out=ad[:, :], in0=e_bf[:, :],
                                s0=xs[:, b:b + 1],
                            )
                            r.ins.perf_max = 3
                        ad_tiles.append(ad)
                    for j in range(4):
                        lhsT = W[:, 31 - q:63 - q]
                        for c in range(N_CHUNKS):
                            nc.tensor.matmul(
                                out=psum_t[32 * j:32 * j + 32, 512 * c:512 * (c + 1)],
                                lhsT=lhsT,
                                rhs=ad_tiles[j][:, 512 * c:512 * (c + 1)],
                                start=False,
                                stop=False,
                                tile_position=(0, 32 * j),
                            )
                # Close each bank's accumulation group with a full-partition
                # zero matmul (adds 0; start=False keeps data, stop=True
                # closes the group over all 128 partitions). N=1: the stop
                # applies to the whole bank region.
                for c in range(N_CHUNKS):
                    nc.tensor.matmul(
                        out=psum_t[:, 512 * c:512 * c + 1],
                        lhsT=Z[:, :],
                        rhs=e_bf[:, 0:1],
                        start=False,
                        stop=True,
                        tile_position=(0, 0),
                    )

                out_t = opool.tile([128, NE], F32, tag="ot")
                for c in range(N_CHUNKS):
                    nc.scalar.activation(
                        out=out_t[:, 512 * c:512 * (c + 1)],
                        in_=psum_t[:, 512 * c:512 * (c + 1)],
                        func=AF.Exp, bias=0.0, scale=-C,
                    )
                    # Alternate the issuing engine: descriptor generation is
                    # ~0.7us on the issuing sequencer.
                    (nc.sync if c % 2 == 0 else nc.gpsimd).dma_start(
                        out=out[128 * g:128 * (g + 1), 512 * c:512 * (c + 1)],
                        in_=out_t[:, 512 * c:512 * (c + 1)],
                    )

    nc.compile()
    return nc


_NC_CACHE = None


def _get_nc():
    global _NC_CACHE
    if _NC_CACHE is None:
        _NC_CACHE = build_nc()
    return _NC_CACHE


def make_in_maps(inputs, exemplars, attn):
    eT = np.ascontiguousarray(exemplars.astype(np.float32).T)          # (128, 2048)
    at = np.ascontiguousarray(attn.astype(np.float32).reshape(ND, 1))  # (128, 1)
    in_maps = []
    for i in range(N_CORES):
        xT = np.ascontiguousarray(
            inputs[i * BL:(i + 1) * BL].astype(np.float32).T
        )  # (128, 256)
        in_maps.append({"xT": xT, "eT": eT, "attn": at})
    return in_maps


def run(inputs, exemplars, attn, trace=False, **kwargs):
    nc = _get_nc()
    in_maps = make_in_maps(np.asarray(inputs), np.asarray(exemplars), np.asarray(attn))
    res = run_bass_kernel_spmd(
        nc, in_maps, core_ids=list(range(N_CORES)), trace=trace, **kwargs
    )
    out = np.concatenate([res.results[i]["out"] for i in range(N_CORES)], axis=0)
    return out, res


def kernel(inputs, exemplars, attn):
    out, _ = run(inputs, exemplars, attn, trace=False)
    return out


# revision 47
# speedup vs baseline: 1.0687x; 1.0082x over previous
"""ALCOVE RBF similarity kernel for Trainium2 (8 NeuronCores, data-parallel).

Computes out[b, e] = exp(-C * sum_d attn[d] * |exemplars[e, d] - inputs[b, d]|)
for inputs (2048, 128), exemplars (2048, 128), attn (128,) -> out (2048, 2048).

Strategy (per core, batch shard of 256 rows):
  - Layout: d (=128) on SBUF partitions, ne (=2048) on the free axis.
  - Pre-scale exemplars/inputs by attn on device (attn_d * |e-x| == |attn_d*e - attn_d*x|
    since attn >= 0), exemplars cast to bf16.
  - Abs-diff tiles |e'^T - x'_b| (128 x 2048 bf16, one per batch row) are
    produced by BOTH elementwise engines, balanced so they finish together:
      * VectorE (~74%): a custom DVE op (registered below) that runs the
        cayman float ABSOLUTE_DIFF ALU op with the stock tensor_scalar
        perf-mode programs -> 4x bf16 rate, one op per row (~663ns).
      * ScalarE (~26%): activation(Abs, bias=-x'_b per partition) (~1894ns).
  - TensorE: reduce over d (partitions) via matmul with a ones-column
    stationary. 4-way column tiling (tile_position=(0, 32j)) runs 4 batch
    rows concurrently; a sliding one-hot window in the stationary places row
    b at PSUM partition b % 128, accumulating over 32 quads into a
    (128, 2048) f32 PSUM tile (opened/closed by zero-matmuls carrying
    start/stop).
  - ScalarE: exp(-C * dist) fused via activation scale, PSUM -> SBUF, chunked
    so output DMA overlaps.
  - DMA issue is spread across the idle sync/gpsimd sequencers (descriptor
    generation costs ~0.7us per dma_start on the issuing engine).
  - Whole-chip: 8-way batch-parallel, no collectives; host transposes inputs
    and concatenates the 8 output shards.

Measured on TRN2: ~155us NEFF exec (fast clock state; the chip sometimes
sits in a ~1.2x slower power state), rel err ~9e-4 vs the f32 reference.
VectorE/ScalarE are both >95% busy in steady state; TensorE ~27%.
"""

import numpy as np

import concourse.bacc as bacc
import concourse.tile as tile
from concourse import mybir
from concourse.bass_utils import run_bass_kernel_spmd


def _register_absdiff_op():
    """Register a custom DVE op TS_ABS_DIFF_ANT: out = |in0 - s0| in ONE
    VectorE instruction with full 1x/2x/4x perf-mode programs.

    The stock TENSOR_SCALAR uop programs (table slots 16..19 on gen3) are
    op-agnostic: their datapath stages use INSTRUCTION_OP_0/1 (wire 32/33),
    taking the ALU op from the instruction bytes. The cayman ISA has a float
    ABSOLUTE_DIFF ALU op (wire 23), but bass's AluOpType cannot express it.
    So we clone the stock programs with the ALU op hardwired to
    ABSOLUTE_DIFF (op1 stage -> BYPASS) and register them as a custom DVE op
    with perf_max=3, inheriting the stock modes' packing/port wiring.
    """
    import concourse.dve_ops as dve_ops
    from concourse.dve_spec import Spec, Src0, C0, Bin, AluOp as DAluOp
    from concourse.dve_uop import DveOpSpec
    from concourse.dve_tables import find_stock_dve_bin_dir, load_table_set

    NAME = "TS_ABS_DIFF_ANT"
    for op in dve_ops.OPS:
        if op.name == NAME:
            return op

    ts = load_table_set(find_stock_dve_bin_dir("gen3"), "default", "v3")

    class RawUop:
        next_uop = (0, 0, 0)

        def __init__(self, slot):
            self.cf = dict(ts.control_fast[slot])
            self.cs = dict(ts.control_slow[slot])
            self.dp = [dict(b) for b in ts.datapath[slot]]
            for b in self.dp:
                if b.get("alu_op") == 32:      # INSTRUCTION_OP_0 -> ABSOLUTE_DIFF
                    b["alu_op"] = 23
                elif b.get("alu_op") == 33:    # INSTRUCTION_OP_1 -> BYPASS
                    b["alu_op"] = 0

        def to_entries(self, ver):
            return (self.cf, self.cs, self.dp)

        def validate(self, ver):
            pass

    spec = Spec(
        body=Bin(DAluOp.ABSOLUTE_DIFF, Src0, C0),
        reference=lambda in0, in1, s0, s1, imm2: np.abs(in0 - s0),
    )

    row = dve_ops._CUSTOM_DVE_ROW_BASE + len(dve_ops.OPS)
    assert row < 0x20
    compiled = DveOpSpec(
        name=NAME,
        opcode=row,
        uops=[RawUop(16)],
        uops_2x=[RawUop(17)],
        uops_2x_2p=[RawUop(18)],
        uops_4x=[RawUop(19)],
        perf_max=3,
        rd1_en=False,
    )

    class AbsDiffOp:
        pass

    op = AbsDiffOp()
    op.name = NAME
    op.spec = spec
    op.subdim = False
    op.compile = lambda ver, _c=compiled: _c
    dve_ops.OPS.append(op)
    dve_ops._SUB_OPCODE_FOR_NAME[NAME] = row
    dve_ops.CUSTOM_DVE_SPECS[NAME] = spec
    return op


ABS_DIFF_OP = _register_absdiff_op()

N_CORES = 8
BATCH = 2048
NE = 2048
ND = 128
BL = BATCH // N_CORES  # 256 batch rows per core
C = 6.5

F32 = mybir.dt.float32
BF16 = mybir.dt.bfloat16
AF = mybir.ActivationFunctionType
OP = mybir.AluOpType

N_GROUPS = BL // 128          # 2 groups of 128 batch rows
N_QUADS = 32                  # quads per group (4 rows each)
N_CHUNKS = NE // 512          # 4 matmul chunks of 512 cols

# Fraction of batch rows whose abs-diff tile is produced on ScalarE
# (activation Abs with per-partition bias, measured ~1894ns) vs VectorE
# (custom fused |in0 - s0| at 4x bf16, measured ~663ns). Balanced so both
# engines finish together (ScalarE also runs the exp epilogues).
ACT_FRACTION = 0.275  # 66 of 240 eligible rows on ScalarE (measured balance)


def build_nc():
    nc = bacc.Bacc(
        "TRN2", target_bir_lowering=False, debug=False, num_devices=N_CORES
    )

    xT = nc.dram_tensor("xT", [ND, BL], F32, kind="ExternalInput")
    eT = nc.dram_tensor("eT", [ND, NE], F32, kind="ExternalInput")
    attn = nc.dram_tensor("attn", [ND, 1], F32, kind="ExternalInput")
    out = nc.dram_tensor("out", [BL, NE], F32, kind="ExternalOutput")

    with tile.TileContext(nc) as tc:
        with (
            tc.tile_pool(name="const", bufs=1) as cpool,
            tc.tile_pool(name="adp", bufs=14) as adpool,
            tc.tile_pool(name="psum", bufs=2, space="PSUM") as ppool,
            tc.tile_pool(name="outp", bufs=2) as opool,
        ):
            eT_sb = cpool.tile([ND, NE], F32)
            e_bf = cpool.tile([ND, NE], BF16)
            xT_sb = cpool.tile([ND, BL], F32)
            xs = cpool.tile([ND, BL], F32)      # attn-scaled inputs (f32)
            nxs = cpool.tile([ND, BL], F32)     # -attn-scaled inputs (bias form)
            at_sb = cpool.tile([ND, 1], F32)
            W = cpool.tile([ND, 63], BF16)      # sliding one-hot ones column
            Z = cpool.tile([ND, 128], BF16)     # all-zero stationary (PSUM clear)

            # Spread the 1MB exemplar load over DMA queues AND over three
            # issuing engines: descriptor generation costs ~0.7us per
            # dma_start on the issuing sequencer, so a single engine would
            # serialize ~11us of issue latency.
            issuers = [nc.sync, nc.gpsimd]
            nc.gpsimd.dma_start(out=at_sb[:, :], in_=attn[:, :])
            nc.sync.dma_start(out=xT_sb[:, :], in_=xT[:, :])
            for c in range(12):
                issuers[c % 2].dma_start(
                    out=eT_sb[:, 170 * c:170 * (c + 1) if c < 11 else NE],
                    in_=eT[:, 170 * c:170 * (c + 1) if c < 11 else NE],
                )

            nc.vector.memset(W[:, :], 0.0)
            nc.vector.memset(W[:, 31:32], 1.0)
            nc.vector.memset(Z[:, :], 0.0)

            # Prime the ScalarE activation tables (Exp set, which also holds
            # Abs) while the input DMAs are still in flight, so the ~2.7us
            # table load is off the critical path.
            prime = cpool.tile([ND, 1], F32)
            nc.vector.memset(prime[:, :], 0.0)
            nc.scalar.activation(
                out=prime[:, :], in_=prime[:, :], func=AF.Exp,
                bias=0.0, scale=1.0,
            )
            nc.scalar.activation(
                out=prime[:, :], in_=prime[:, :], func=AF.Abs,
                bias=0.0, scale=1.0,
            )

            # Prime the custom-DVE uop table the same way: a dummy fused
            # abs-diff op issued before the inputs arrive pulls the per-NEFF
            # DVE table load off the critical path.
            vprime = cpool.tile([ND, 2], BF16)
            vprime_s = cpool.tile([ND, 1], F32)
            nc.vector.memset(vprime[:, :], 0.0)
            nc.vector.memset(vprime_s[:, :], 0.0)
            r = nc.vector._custom_dve(
                ABS_DIFF_OP, out=vprime[:, :], in0=vprime[:, :],
                s0=vprime_s[:, 0:1],
            )
            r.ins.perf_max = 3

            # x' = attn * x (f32) and -x' first: they only need the tiny xT
            # DMA, so ScalarE's first abs-diff isn't gated on the big load.
            nc.vector.tensor_scalar(
                out=xs[:, :], in0=xT_sb[:, :], scalar1=at_sb[:, 0:1],
                scalar2=None, op0=OP.mult,
            )
            nc.vector.tensor_scalar(
                out=nxs[:, :], in0=xs[:, :], scalar1=-1.0,
                scalar2=None, op0=OP.mult,
            )
            # e' = attn * e (bf16)
            for c in range(4):
                nc.vector.tensor_scalar(
                    out=e_bf[:, 512 * c:512 * (c + 1)],
                    in0=eT_sb[:, 512 * c:512 * (c + 1)],
                    scalar1=at_sb[:, 0:1],
                    scalar2=None, op0=OP.mult,
                )

            for g in range(N_GROUPS):
                psum_t = ppool.tile([128, NE], F32)
                # Clear each PSUM bank: full-partition zero matmul opens the
                # accumulation group (start=True zeroes has_written + data).
                for c in range(N_CHUNKS):
                    nc.tensor.matmul(
                        out=psum_t[:, 512 * c:512 * (c + 1)],
                        lhsT=Z[:, :],
                        rhs=e_bf[:, 0:512],
                        start=True,
                        stop=False,
                        tile_position=(0, 0),
                    )
                for q in range(N_QUADS):
                    ad_tiles = []
                    for j in range(4):
                        b = 128 * g + 32 * j + q
                        ad = adpool.tile([ND, NE], BF16, tag="ad")
                        # Give the last quads of each group to VectorE so the
                        # exp epilogue isn't gated by a late ScalarE abs-diff.
                        k = 128 * g + 4 * q + j
                        act_cnt = (
                            int((k + 1) * ACT_FRACTION) - int(k * ACT_FRACTION)
                            if k % 128 < 120 else 0
                        )
                        if act_cnt:
                            # ScalarE: |e' - x'_b| = Abs(e' + (-x'_b))
                            nc.scalar.activation(
                                out=ad[:, :], in_=e_bf[:, :], func=AF.Abs,
                                bias=nxs[:, b:b + 1], scale=1.0,
                            )
                        else:
                            # VectorE: custom fused |e' - x'_b| (ABSOLUTE_DIFF
                            # ALU op) with stock tensor_scalar perf modes.
                            r = nc.vector._custom_dve(
                                ABS_DIFF_OP, out=ad[:, :], in0=e_bf[:, :],
                                s0=xs[:, b:b + 1],
                            )
                            r.ins.perf_max = 3
                        ad_tiles.append(ad)
                    for j in range(4):
                        lhsT = W[:, 31 - q:63 - q]
                        for c in range(N_CHUNKS):
                            nc.tensor.matmul(
                                out=psum_t[32 * j:32 * j + 32, 512 * c:512 * (c + 1)],
                                lhsT=lhsT,
                                rhs=ad_tiles[j][:, 512 * c:512 * (c + 1)],
                                start=False,
                                stop=False,
                                tile_position=(0, 32 * j),
                            )
                # Close each bank's accumulation group with a full-partition
                # zero matmul (adds 0; start=False keeps data, stop=True
                # closes the group over all 128 partitions). N=1: the stop
                # applies to the whole bank region.
                for c in range(N_CHUNKS):
                    nc.tensor.matmul(
                        out=psum_t[:, 512 * c:512 * c + 1],
                        lhsT=Z[:, :],
                        rhs=e_bf[:, 0:1],
                        start=False,
                        stop=True,
                        tile_position=(0, 0),
                    )

                out_t = opool.tile([128, NE], F32, tag="ot")
                for c in range(2):
                    nc.scalar.activation(
                        out=out_t[:, 1024 * c:1024 * (c + 1)],
                        in_=psum_t[:, 1024 * c:1024 * (c + 1)],
                        func=AF.Exp, bias=0.0, scale=-C,
                    )
                    # Alternate the issuing engine: descriptor generation is
                    # ~0.7us on the issuing sequencer.
                    (nc.sync if c % 2 == 0 else nc.gpsimd).dma_start(
                        out=out[128 * g:128 * (g + 1), 1024 * c:1024 * (c + 1)],
                        in_=out_t[:, 1024 * c:1024 * (c + 1)],
                    )

    nc.compile()
    return nc


_NC_CACHE = None


def _get_nc():
    global _NC_CACHE
    if _NC_CACHE is None:
        _NC_CACHE = build_nc()
    return _NC_CACHE


def make_in_maps(inputs, exemplars, attn):
    eT = np.ascontiguousarray(exemplars.astype(np.float32).T)          # (128, 2048)
    at = np.ascontiguousarray(attn.astype(np.float32).reshape(ND, 1))  # (128, 1)
    in_maps = []
    for i in range(N_CORES):
        xT = np.ascontiguousarray(
            inputs[i * BL:(i + 1) * BL].astype(np.float32).T
        )  # (128, 256)
        in_maps.append({"xT": xT, "eT": eT, "attn": at})
    return in_maps


def run(inputs, exemplars, attn, trace=False, **kwargs):
    nc = _get_nc()
    in_maps = make_in_maps(np.asarray(inputs), np.asarray(exemplars), np.asarray(attn))
    res = run_bass_kernel_spmd(
        nc, in_maps, core_ids=list(range(N_CORES)), trace=trace, **kwargs
    )
    out = np.concatenate([res.results[i]["out"] for i in range(N_CORES)], axis=0)
    return out, res


def kernel(inputs, exemplars, attn):
    out, _ = run(inputs, exemplars, attn, trace=False)
    return out
